# revision 3
# baseline (speedup 1.0000x reference)
"""Trainium2 Bass kernel for nn_ChamferDistance_sumknn (B=1, N=M=8192, D=3, K=4).

Strategy
--------
All heavy work (three 8192x8192 squared-distance matrices + their reductions)
runs on 8 NeuronCores, sharded by row-block (each core owns 1024 rows of the
j/n dimension with the full opposite extent, so no cross-core collectives are
needed):

  stripe 1 (Dcd, Y-major):  psum[j,n] = -(X2[n]+Y2[j]-2 x.y); per 2048-chunk
            Max8 + MaxIndex give the 8 smallest distances + indices (column
            argmin candidates).
  stripe 2 (Dyy, Y-major):  same, gives per-column top-8 nearest-neighbor
            candidates of Y among Y.
  stripe 3 (Dcd, X-major):  per-chunk free-dim max of -D gives row minima.

The distance values are produced in a single fp32r matmul per tile with a
K=13 augmented contraction: operands are split into hi/lo parts with <=12-bit
significands (exactly representable in the PE's FP22 datapath), so the psum
result has full fp32-grade accuracy (measured max err ~7.6e-6 vs fp64 on HW).

The host then re-evaluates the <=32 candidates per column with arithmetic that
is bit-identical to the jax-CPU reference (fma-based dot), so the argmin /
top-4 selections match the reference exactly; the tiny O(K*M) tail of the
computation (gathers, Dknn sum) is the unshard/assembly glue.
"""

import os
import numpy as np
from contextlib import ExitStack

B, N, M, D, TOPK = 1, 8192, 8192, 3, 4
CORES = 8
JS = N // CORES          # 1024 rows per core
NB = JS // 128           # 8 partition-blocks per core
CH = 2048                # scan chunk (free dim)
NCH = M // CH            # 4 chunks per full row
KAUG = 13                # augmented contraction length
INW = 2 * JS + 2 * M     # input tensor columns: Wcd | Wcx | MX | MY
OUTW = 132               # cd_vals(32) cd_idx(32) dy_vals(32) dy_idx(32) row(4)

f32 = np.float32
f64 = np.float64

# ----------------------------------------------------------------- host math

def _split_hilo(a):
    a = np.ascontiguousarray(a, dtype=f32)
    hi = (a.view(np.uint32) & np.uint32(0xFFFFF000)).view(f32)
    lo = (a - hi).astype(f32)
    return hi, lo


def _norms(P):
    P = P.astype(f32)
    return ((P[:, 0] * P[:, 0] + P[:, 1] * P[:, 1]) + P[:, 2] * P[:, 2]).astype(f32)


def _weights_form(P, P2, negate):
    ph, pl = _split_hilo(P)
    p2h, p2l = _split_hilo(P2)
    ones = np.ones(P.shape[0], f32)
    W = np.stack([ph[:, 0], ph[:, 1], ph[:, 2],
                  pl[:, 0], pl[:, 1], pl[:, 2],
                  ph[:, 0], ph[:, 1], ph[:, 2],
                  p2h, p2l, ones, ones], axis=0)
    return np.ascontiguousarray(-W if negate else W, f32)


def _moving_form(Q, Q2):
    qh, ql = _split_hilo(Q)
    n2 = f32(-2.0)
    qh2 = n2 * qh
    ql2 = n2 * ql
    q2h, q2l = _split_hilo(Q2)
    ones = np.ones(Q.shape[0], f32)
    Mv = np.stack([qh2[:, 0], qh2[:, 1], qh2[:, 2],
                   qh2[:, 0], qh2[:, 1], qh2[:, 2],
                   ql2[:, 0], ql2[:, 1], ql2[:, 2],
                   ones, ones, q2h, q2l], axis=0)
    return np.ascontiguousarray(Mv, f32)


def _fma(a, b, c):
    return (a.astype(f64) * b.astype(f64) + c.astype(f64)).astype(f32)


def _pair_dist_exact(Pg, Qg, P2g, Q2g):
    """Bit-identical to the jax-CPU reference pairwise_sq on gathered points:
    (P2+Q2) - 2*fma_dot(p,q) with dot = fma(x2,y2, fma(x1,y1, x0*y0))."""
    d0 = (Pg[..., 0] * Qg[..., 0]).astype(f32)
    d1 = _fma(Pg[..., 1], Qg[..., 1], d0)
    e = _fma(Pg[..., 2], Qg[..., 2], d1)
    t = (P2g + Q2g).astype(f32)
    return t - f32(2.0) * e

# -------------------------------------------------------------- bass program

def _patch_tile_drain():
    """This walrus build allows very few sync-wait commands per instruction;
    Tile's kernel-tail drain aggregates one wait per live processor onto a
    single Drain and overflows the budget. Split into one drain per wait."""
    from concourse import tile
    from concourse.vector_clock import ScopedClock, VectorClock

    if getattr(tile.TileContext, "_chamfer_drain_patch", False):
        return
    tile.TileContext._chamfer_drain_patch = True

    def _drain_and_barrier(self, tick_clock, wait_clock):
        nc = self.nc
        vc = tick_clock.global_clock
        for proc in range(64):
            try:
                cur = vc.peek_next(proc) - 1
            except Exception:
                break
            if cur <= 0:
                continue
            single = VectorClock()
            single.require_at_least(proc, cur)
            d = nc.sync.drain()
            wait_clock.add_sem_waits(d.ins, ScopedClock({None: single}))
        nc.all_engine_barrier()
        assert self.sems is not None
        popped = nc._tile_sem_poison_stack.pop()
        assert popped is self._sem_poison
        nc.clear_and_free_semaphores(list(self.sems.allocated().values()))
        nc.all_engine_barrier()

    tile.TileContext._drain_and_barrier = _drain_and_barrier


def _split_excess_waits(nc):
    """Walrus on this image rejects instructions carrying more than a tiny
    number of sync-wait commands (Matmult/DMACopy/Drain tolerate just one).
    Move excess waits onto preceding same-engine NoOps — engines execute
    in order, so a NoOp that waits provides the same guarantee."""
    import concourse.mybir as mybir

    n_split = 0
    for fn in nc.m.functions:
        for blk in fn.blocks:
            new = []
            for ins in blk.instructions:
                si = ins.sync_info
                waits = list(si.on_wait) if si is not None and si.on_wait else []
                cap = 1
                if len(waits) > cap:
                    for w in waits[:-cap]:
                        n_split += 1
                        nop = mybir.InstNoOp(
                            name=f"{ins.name}-wsplit{n_split}", ins=[], outs=[])
                        nop.engine = ins.engine
                        nop.sync_info = mybir.SyncInfo(on_wait=[w], on_update=[])
                        new.append(nop)
                    ins.sync_info = mybir.SyncInfo(
                        on_wait=waits[-cap:],
                        on_update=list(si.on_update) if si.on_update else [])
                new.append(ins)
            blk.instructions = new
    return n_split


def _build_program():
    import concourse.bass as bass
    import concourse.mybir as mybir
    from concourse.tile import TileContext

    _patch_tile_drain()

    nc = bass.Bass("TRN2", debug=False, num_devices=CORES)
    in_all = nc.dram_tensor("in_all", [KAUG, INW], mybir.dt.float32r,
                            kind="ExternalInput")
    out_all = nc.dram_tensor("out_all", [JS, OUTW], mybir.dt.float32,
                             kind="ExternalOutput")

    with TileContext(nc) as tc, ExitStack() as ctx:
        sb = ctx.enter_context(tc.tile_pool(name="sb", bufs=1))
        scan_pool = ctx.enter_context(tc.tile_pool(name="scan", bufs=3))
        out_pool = ctx.enter_context(tc.tile_pool(name="outp", bufs=2))
        ps = ctx.enter_context(tc.tile_pool(name="ps", bufs=2, space="PSUM"))

        wm = sb.tile([KAUG, INW], mybir.dt.float32r)
        nc.gpsimd.dma_start(wm[:], in_all[:, :])
        Wcd = wm[:, 0:JS]
        Wcx = wm[:, JS:2 * JS]
        MX = wm[:, 2 * JS:2 * JS + M]
        MY = wm[:, 2 * JS + M:2 * JS + 2 * M]

        for jb in range(NB):
            ot = out_pool.tile([128, OUTW], mybir.dt.float32)
            # stripes 1+2: Y-major Dcd (rhs=MX) and Dyy (rhs=MY)
            for si, rhs in ((0, MX), (1, MY)):
                w = Wcd[:, jb * 128:(jb + 1) * 128]
                for ck in range(NCH):
                    pt = ps.tile([128, CH], mybir.dt.float32, tag="ps")
                    for t in range(CH // 512):
                        nc.tensor.matmul(
                            out=pt[:, t * 512:(t + 1) * 512],
                            lhsT=w,
                            rhs=rhs[:, ck * CH + t * 512: ck * CH + (t + 1) * 512],
                            start=True, stop=True)
                    scan = scan_pool.tile([128, CH], mybir.dt.float32, tag="scan")
                    nc.scalar.copy(out=scan[:], in_=pt[:])
                    co = si * 64 + ck * 8
                    nc.vector.max(out=ot[:, co:co + 8], in_=scan[:])
                    nc.vector.max_index(
                        out=ot[:, 32 + co:40 + co].bitcast(mybir.dt.uint32),
                        in_max=ot[:, co:co + 8], in_values=scan[:])
            # stripe 3: X-major Dcd row minima (lhsT from Wcx, rhs=MY)
            w = Wcx[:, jb * 128:(jb + 1) * 128]
            for ck in range(NCH):
                pt = ps.tile([128, CH], mybir.dt.float32, tag="ps")
                for t in range(CH // 512):
                    nc.tensor.matmul(
                        out=pt[:, t * 512:(t + 1) * 512],
                        lhsT=w,
                        rhs=MY[:, ck * CH + t * 512: ck * CH + (t + 1) * 512],
                        start=True, stop=True)
                nc.vector.tensor_reduce(
                    out=ot[:, 128 + ck:129 + ck], in_=pt[:],
                    axis=mybir.AxisListType.X, op=mybir.AluOpType.max)
            nc.gpsimd.dma_start(out_all[jb * 128:(jb + 1) * 128, :], ot[:])
    _split_excess_waits(nc)
    return nc


_PROGRAM_CACHE = {}


def _get_program():
    if "nc" not in _PROGRAM_CACHE:
        _PROGRAM_CACHE["nc"] = _build_program()
    return _PROGRAM_CACHE["nc"]

# ------------------------------------------------------------------- kernel

def kernel(X, Y, kn, Dy, _collect_timing=None):
    from concourse.bass_utils import run_bass_kernel_spmd

    Xs = np.ascontiguousarray(np.asarray(X), f32)[0]   # [N,3]
    Ys = np.ascontiguousarray(np.asarray(Y), f32)[0]   # [M,3]
    X2 = _norms(Xs)
    Y2 = _norms(Ys)

    W_Y = _weights_form(Ys, Y2, negate=True)   # [13, M]
    W_X = _weights_form(Xs, X2, negate=True)   # [13, N]
    M_X = _moving_form(Xs, X2)                 # [13, N]
    M_Y = _moving_form(Ys, Y2)                 # [13, M]

    in_maps = []
    for c in range(CORES):
        sl = slice(c * JS, (c + 1) * JS)
        in_maps.append({"in_all": np.ascontiguousarray(
            np.concatenate([W_Y[:, sl], W_X[:, sl], M_X, M_Y], axis=1))})

    nc = _get_program()
    kwargs = {}
    if _collect_timing is not None:
        kwargs = dict(_collect_timing)
    res = run_bass_kernel_spmd(nc, in_maps, core_ids=list(range(CORES)), **kwargs)
    outs = [res.results[c]["out_all"] for c in range(CORES)]
    if _collect_timing is not None:
        _collect_timing["result"] = res

    out = np.concatenate(outs, axis=0)          # [N, OUTW]

    # ---- row (Dr) term: min over j per row n, value only
    rowmin = -out[:, 128:132].max(axis=1)
    Dr = np.mean(rowmin, dtype=f32)

    # ---- candidate assembly helpers
    chunk_off = np.repeat(np.arange(NCH, dtype=np.uint32) * CH, 8)[None, :]  # [1,32]

    def select(vals_cols, idx_cols, opp_pts, opp_norms, own_pts, own_norms, k):
        """Exact re-selection of the k smallest per column among candidates."""
        cvals = -out[:, vals_cols]                       # [N, 32] device values
        cidx = out[:, idx_cols].view(np.uint32) + chunk_off   # [N, 32] global idx
        del cvals  # selection is redone with exact values; device vals unused
        own_idx = np.arange(N)
        d_exact = _pair_dist_exact(
            opp_pts[cidx], own_pts[own_idx][:, None, :],
            opp_norms[cidx], own_norms[own_idx][:, None])     # [N, 32]
        # order by (value, index) ascending — matches argmin/top_k tie-breaks
        ordr = np.lexsort((cidx, d_exact), axis=1)[:, :k]
        rows = own_idx[:, None]
        return d_exact[rows, ordr], cidx[rows, ordr]

    # ---- column (Dc) term + assignment indices from Dcd stripe
    cd_vals, cd_idx = select(slice(0, 32), slice(32, 64), Xs, X2, Ys, Y2, 1)
    Dc = np.mean(cd_vals[:, 0], dtype=f32)
    indc = cd_idx[:, 0].astype(np.int64)                 # [M]

    # ---- Dyy top-4 from Dyy stripe
    dy_vals, dy_idx = select(slice(64, 96), slice(96, 128), Ys, Y2, Ys, Y2, TOPK)
    kn_idx = dy_idx.astype(np.int64)                     # [M, 4] ranks 0..3
    dists_y = dy_vals                                    # [M, 4]

    # ---- Dknn: dists_x over gathered XX = X[indc]
    XX = Xs[indc]                                        # [M, 3]
    XX2 = _norms(XX)
    Xi = XX[kn_idx]                                      # [M, 4, 3]
    X2i = XX2[kn_idx]                                    # [M, 4]
    dists_x = _pair_dist_exact(Xi, XX[:, None, :], X2i, XX2[:, None])  # [M,4]
    diff = (dists_x[:, 1:] - dists_y[:, 1:]).astype(f32)
    Dk = np.sum(diff * diff, axis=1, dtype=f32)          # [M]
    Dknn = np.sum(Dk, dtype=f32)

    d_ch = f32(Dr + Dc)
    return (np.array([d_ch], f32), np.array([Dknn], f32))
